# revision 31
# baseline (speedup 1.0000x reference)
"""Trainium2 Bass kernel for 16->16 channel 3x3 VALID conv on [16,1536,1536].

out[co, y, x] = sum_{ci,dy,dx} W[co,ci,dy,dx] * X[ci, y+dy, x+dx] + sum(bias)

Strategy (8-core data parallel over H, halo of 2 rows):
  Each core computes 192 output rows from a 194-row input shard, in 32 blocks
  of R=6 output rows. Per block, an SBUF "window" of 8 input rows x 16
  channels is laid out as [128, 1536] with partition p = k*16 + ci (k = row
  in window, k-major). The conv becomes 3 accumulating matmuls (one per
  kernel column dx) whose dx shift is a free-dim offset on the moving
  operand:
     psum[m=(co*6+r), x] += lhsT_dx[p, m] * window[p, x+dx]
  with block-Toeplitz weights lhsT_dx[k*16+ci, co*6+r] = W[co,ci,k-r,dx]
  (zero outside 0 <= k-r <= 2), precomputed on host from the 9KB weight.
  Contraction K=128, M=96 -> 3 column-streams per 6 output pixels (0.5
  PE-cycles/output-pixel).

  Matmuls run in float32r (tf32-like, 1 col/cycle at N>=256; fp32 would be
  4x slower). The rounding to f32r happens in a Vector-engine copy; the
  verifier requires a rounding producer for f32r matmul inputs.

  HBM traffic: the 2 halo rows shared by consecutive blocks are carried
  across in SBUF (window k=6,7 -> next window k=0,1 via a DVE copy), so each
  input row is read exactly once. Input DMAs issue on the sync-engine HWDGE
  queue, output DMAs on the scalar-engine queue to avoid queue serialization.
"""

import numpy as np

import concourse.bass as bass
import concourse.mybir as mybir
import concourse.tile as tile
from concourse.bass_utils import run_bass_kernel_spmd

C = 16
H = 1536
W = 1536
HOUT = H - 2
WOUT = W - 2
NCORES = 8
ROWS_PER_CORE = 192  # output rows computed per core
R = 6                # output rows per block
WIN = R + 2          # input rows per window
NBLK = ROWS_PER_CORE // R
XIN_ROWS = ROWS_PER_CORE + 2  # input rows per shard
CHUNKS = [(0, 512), (512, 512), (1024, WOUT - 1024)]
F32 = mybir.dt.float32
F32R = mybir.dt.float32r
F16 = mybir.dt.float16

_drain_patched = False


def _patch_tile_drain():
    """This container's walrus accepts only ONE sync-wait per lowered
    instruction (CTRL drains, S3_LW weight loads, ...). Tile freely attaches
    several. Split the extras onto single-wait nops placed just before the
    instruction on the same engine (identical blocking semantics)."""
    global _drain_patched
    if _drain_patched:
        return
    _drain_patched = True
    from concourse.tile import ScopedClock

    def _split_multi_waits(ordered):
        for bb_name, insts in ordered.items():
            out = []
            for inst in insts:
                si = getattr(inst, "sync_info", None)
                if (
                    si is not None
                    and si.on_wait is not None
                    and len(si.on_wait) > 1
                    and type(inst).__module__ == "bass_rust"
                ):
                    waits = list(si.on_wait)
                    for i, w in enumerate(waits[:-1]):
                        out.append(
                            mybir.InstNoOp(
                                name=f"{inst.name}ws{i}",
                                engine=inst.engine,
                                bass_nofuse=True,
                                sync_info=mybir.SyncInfo(
                                    on_wait=[w], on_update=[]
                                ),
                            )
                        )
                    inst.sync_info = mybir.SyncInfo(
                        on_wait=[waits[-1]],
                        on_update=list(si.on_update or []),
                    )
                out.append(inst)
            ordered[bb_name] = out
        return ordered

    orig_lower = tile.TileContext._lower_ordered_insts

    def _lower_ordered_insts(self, ordered):
        return orig_lower(self, _split_multi_waits(ordered))

    tile.TileContext._lower_ordered_insts = _lower_ordered_insts

    def _drain_and_barrier(self, tick_clock, wait_clock):
        drain_inst = self.nc.sync.drain()
        wait_clock.add_sem_waits(
            drain_inst.ins, ScopedClock({None: tick_clock.global_clock})
        )
        si = drain_inst.ins.sync_info
        if si is not None and si.on_wait is not None and len(si.on_wait) > 1:
            waits = list(si.on_wait)
            drain_inst.ins.sync_info = mybir.SyncInfo(
                on_wait=[waits[0]], on_update=list(si.on_update or [])
            )
            for w in waits[1:]:
                n = self.nc.sync.nop(nofuse=True, hint="drain_wait_split")
                n.ins.sync_info = mybir.SyncInfo(on_wait=[w], on_update=[])
        self.nc.all_engine_barrier()
        assert self.sems is not None
        popped = self.nc._tile_sem_poison_stack.pop()
        assert popped is self._sem_poison
        self.nc.clear_and_free_semaphores(list(self.sems.allocated().values()))
        self.nc.all_engine_barrier()

    tile.TileContext._drain_and_barrier = _drain_and_barrier


def build_lhsT(weight: np.ndarray) -> np.ndarray:
    """[C_out=16, C_in=16, 3, 3] -> [3, 128, 96] block-Toeplitz stationary
    operands, one per kernel column dx.
    lhsT[dx, ci*8+k, co*6+r] = weight[co, ci, k-r, dx] for 0 <= k-r <= 2."""
    lhsT = np.zeros((3, 128, 96), np.float32)
    ci = np.arange(C)
    co = np.arange(C)
    for dx in range(3):
        for dy in range(3):
            for r in range(R):
                k = r + dy
                lhsT[dx, (ci * WIN + k)[:, None], (co * R + r)[None, :]] = (
                    weight[:, :, dy, dx].T
                )
    return lhsT


def shard_windows(
    Xs: np.ndarray, group: int, dtype=np.float32
) -> np.ndarray:
    """Host-side window predup for one core's shard [C, XIN_ROWS, W] ->
    [ngroups, 128, group*W] where out[g, ci*8+k, w*W+x] =
    Xs[ci, 6*(group*g+w)+k, x]. dma_start carries a large fixed overhead
    here, so shipping the 33% halo duplication in exchange for one
    contiguous multi-MB DMA per group is a clear win. For 16-bit matmul
    dtypes the cast happens here too, halving the DMA bytes."""
    ngroups = NBLK // group
    rows = (
        R * group * np.arange(ngroups)[:, None, None]
        + R * np.arange(group)[None, :, None]
        + np.arange(WIN)[None, None, :]
    )  # [g, w, k]
    arr = Xs[:, rows, :].astype(dtype)  # [C, g, w, k, W]
    arr = arr.transpose(1, 0, 3, 2, 4)  # [g, C, k, w, W]
    return np.ascontiguousarray(
        arr.reshape(ngroups, 128, group * W)
    )


def build_program(
    bias_sum: float,
    mm_dtype=F32R,
    nblk=NBLK,
    group=4,
    xbufs=3,
    wbufs=2,
    obufs=4,
    pbufs=6,
    repeat=1,
    hw_loop=0,
):
    """One core's program: see module docstring. dma_start carries a large
    fixed overhead in this environment, so inputs arrive as host-preduped
    window groups (one contiguous DMA per `group` blocks) and outputs leave
    as one grouped DMA into a [C, R, nblk, WOUT] device layout the host
    re-transposes. `repeat` (python-unrolled) and `hw_loop` (tc.For_i)
    re-run the whole block sweep, for timing amplification only."""
    nc = bass.Bass("TRN2", target_bir_lowering=False, debug=False)
    assert nblk % group == 0
    ngroups = nblk // group
    # 16-bit matmul dtypes are cast host-side: x/wt ship pre-cast, halving
    # input DMA bytes and skipping the on-chip rounding pass. f32r still
    # needs an on-chip DVE rounding producer.
    host_cast = mybir.dt.size(mm_dtype) == 2
    ship_dtype = mm_dtype if host_cast else F32
    x = nc.dram_tensor(
        "x", [ngroups, 128, group * W], ship_dtype, kind="ExternalInput"
    ).ap()
    wt = nc.dram_tensor(
        "wt", [3, 128, 96], ship_dtype, kind="ExternalInput"
    ).ap()
    # f16 output halves the store-side HBM traffic; the host re-expands to
    # f32. Quantization error (~2^-11 of max|out|) is far inside the 2e-2
    # rel-err budget.
    y = nc.dram_tensor(
        "y", [C, R, nblk, WOUT], F16, kind="ExternalOutput"
    ).ap()
    round_on_chip = (not host_cast) and mm_dtype != F32

    with tile.TileContext(nc) as tc:
        with (
            tc.tile_pool(name="wpool", bufs=1) as wpool,
            tc.tile_pool(name="xpool", bufs=xbufs) as xpool,
            tc.tile_pool(name="winp", bufs=wbufs) as winp,
            tc.tile_pool(name="opool", bufs=obufs) as opool,
            tc.tile_pool(name="ppool", bufs=pbufs, space="PSUM") as ppool,
        ):
            wts = []
            for dx in range(3):
                if round_on_chip:
                    ws = wpool.tile([128, 96], F32, tag=f"ws{dx}", name=f"ws{dx}")
                    nc.sync.dma_start(ws[:], wt[dx])
                    wtile = wpool.tile(
                        [128, 96], mm_dtype, tag=f"w{dx}", name=f"w{dx}"
                    )
                    nc.vector.tensor_copy(wtile[:], ws[:])
                else:
                    wtile = wpool.tile(
                        [128, 96], mm_dtype, tag=f"w{dx}", name=f"w{dx}"
                    )
                    nc.sync.dma_start(wtile[:], wt[dx])
                wts.append(wtile)

            def sweep(prefix):
                for g in range(ngroups):
                    uid = f"{prefix}_{g}"
                    # one contiguous DMA loads `group` preduped 8-row
                    # windows; partition p = ci*8+k, window w at free cols
                    # [w*W, (w+1)*W)
                    if round_on_chip:
                        stage = xpool.tile(
                            [128, group, W], F32, tag="stage", name=f"st{uid}"
                        )
                        nc.sync.dma_start(
                            stage[:],
                            x[g].rearrange("p (w c) -> p w c", w=group),
                        )
                        win = winp.tile(
                            [128, group, W], mm_dtype, tag="win",
                            name=f"win{uid}",
                        )
                        nc.vector.tensor_copy(win[:], stage[:])
                    else:
                        win = winp.tile(
                            [128, group, W], mm_dtype, tag="win",
                            name=f"win{uid}",
                        )
                        nc.sync.dma_start(
                            win[:],
                            x[g].rearrange("p (w c) -> p w c", w=group),
                        )

                    ot = opool.tile(
                        [96, group, W], F16, tag="o", name=f"o_{uid}"
                    )
                    for w in range(group):
                        psums = [
                            ppool.tile(
                                [96, 512], F32, tag="ps", name=f"ps_{uid}_{w}_{i}"
                            )
                            for i in range(len(CHUNKS))
                        ]
                        for dx in range(3):
                            for ic, (x0, n) in enumerate(CHUNKS):
                                nc.tensor.matmul(
                                    psums[ic][:, :n],
                                    wts[dx][:],
                                    win[:, w, x0 + dx : x0 + dx + n],
                                    start=(dx == 0),
                                    stop=(dx == 2),
                                )
                        # PSUM drain + bias + f16 cast on DVE (~680ns per
                        # [96,512] chunk).
                        for ic, (x0, n) in enumerate(CHUNKS):
                            nc.vector.tensor_scalar_add(
                                ot[:, w, x0 : x0 + n],
                                psums[ic][:, :n],
                                float(bias_sum),
                            )
                    # one DMA stores `group` blocks of 6 output rows into the
                    # [C, R, nblk, WOUT] device layout; (blk, x) merge keeps
                    # the dest AP 3-dim. scalar-engine HWDGE queue keeps
                    # outputs off the input queue.
                    nc.scalar.dma_start(
                        y[:, :, g * group : (g + 1) * group, :].rearrange(
                            "c r b x -> c r (b x)"
                        ),
                        ot[:96, :, 0:WOUT],
                    )

            if hw_loop:
                with tc.For_i(
                    0, hw_loop, 1, hint_engines=(mybir.EngineType.PE,)
                ):
                    sweep("L")
            else:
                for rep in range(repeat):
                    sweep(str(rep))
    return nc


def prepare(X: np.ndarray, weight: np.ndarray, bias: np.ndarray):
    """Build (nc, in_maps, starts) for the exact program kernel() runs."""
    X = np.ascontiguousarray(np.asarray(X, dtype=np.float32))
    weight = np.asarray(weight, dtype=np.float32)
    bias = np.asarray(bias, dtype=np.float32)

    _patch_tile_drain()
    import os

    starts = [min(c * ROWS_PER_CORE, H - XIN_ROWS) for c in range(NCORES)]
    if os.environ.get("CONV_V", "2") == "2":
        lhsT = build_lhsT2(weight).astype(np.float16)
        nc = build_program2(float(bias.sum()))
        in_maps = [
            {
                # [194, 16, W] row-major f16 raw shard
                "x": np.ascontiguousarray(
                    X[:, s : s + XIN_ROWS, :].transpose(1, 0, 2)
                ).astype(np.float16),
                "wt": lhsT,
            }
            for s in starts
        ]
        return nc, in_maps, starts

    lhsT = build_lhsT(weight)
    mm_dtype = {
        "f32": F32,
        "f32r": F32R,
        "f16": mybir.dt.float16,
        "bf16": mybir.dt.bfloat16,
    }[os.environ.get("CONV_MM_DTYPE", "f16")]
    group = 4
    host_cast = mybir.dt.size(mm_dtype) == 2
    ship = mybir.dt.np(mm_dtype) if host_cast else np.float32
    wbufs = 6 if host_cast else 2
    nc = build_program(
        float(bias.sum()), mm_dtype, group=group, wbufs=wbufs
    )
    in_maps = [
        {
            "x": shard_windows(X[:, s : s + XIN_ROWS, :], group, ship),
            "wt": lhsT.astype(ship),
        }
        for s in starts
    ]
    return nc, in_maps, starts


def assemble(res_results, starts) -> np.ndarray:
    out = np.empty((C, HOUT, WOUT), np.float32)
    for c in range(NCORES):
        yc = np.asarray(res_results[c]["y"], np.float32)
        if yc.shape[0] == NROUNDS:
            # v2 round-major [NROUNDS, 128, WOUT]: flat row u*32+j with
            # u = 3w+s holding M-rows m = 32s+j of window w (m = co*6+r)
            yc = (
                yc.reshape(NBLK, 3, 32, WOUT)
                .transpose(1, 2, 0, 3)
                .reshape(C, R, NBLK, WOUT)
            )
        out[:, starts[c] : starts[c] + ROWS_PER_CORE, :] = (
            yc.transpose(0, 2, 1, 3).reshape(C, ROWS_PER_CORE, WOUT)
        )
    return out


def build_lhsT2(weight: np.ndarray) -> np.ndarray:
    """k-major block-Toeplitz: lhsT[dx, k*16+ci, co*6+r] = W[co,ci,k-r,dx]
    for 0 <= k-r <= 2. k-major partitions put the window's two halo rows
    (k=6,7) at partitions 96:128 so the next window's halo lands at 0:32
    with a single contiguous partition-shift copy."""
    lhsT = np.zeros((3, 128, 96), np.float32)
    ci = np.arange(C)
    co = np.arange(C)
    for dx in range(3):
        for dy in range(3):
            for r in range(R):
                k = r + dy
                lhsT[dx, (k * 16 + ci)[:, None], (co * R + r)[None, :]] = (
                    weight[:, :, dy, dx].T
                )
    return lhsT


NUNITS = 3 * NBLK  # 96 M=32 col-tile units per core
NROUNDS = NUNITS // 4  # 24 rounds of 4 concurrent units


def build_program2(
    bias_sum: float,
    mm_dtype=F16,
    wbufs=6,
    obufs=6,
    pbufs=8,
    prefetch=4,
):
    """Raw-input, col-tiled program (one core).

    Input x: [194, 16, W] f16 row-major. Window w lives in an SBUF tile
    [128, W], partition p = k*16+ci with k = row-6w. Fresh rows 6w+2..6w+7
    DMA into partitions 32:128 (one contiguous 294KB HBM read); rows
    6w..6w+1 equal window w-1's FRESH rows k=6,7 (partitions 96:128), so a
    single partition-shift DVE copy [32, W] fills 0:32 — halo copies do
    not chain. Input HBM traffic is the raw 9.53MB/core (no 33% predup).

    Matmuls are 4x column-tiled: unit u = 3w+s (s = 32-wide M-slice of
    window w) runs on PE column group g = u%4 via tile_position=(0,32g),
    writing psum quarter [32g:32g+32]. Four units stream concurrently
    through separate XBUSes (HW-probed ~4x), cutting PE time from ~65us
    (M=96, 3/4 of the array) to ~48us. Per round (4 units) there are 3
    chunk psums [128,512] accumulated over dx; drains (+bias, ->f16)
    alternate DVE/ACT per chunk; 2 output DMAs per round go to
    y[96, nblk, WOUT] (m = co*6+r), queues alternating scalar/sync.
    """
    nc = bass.Bass("TRN2", target_bir_lowering=False, debug=False)
    x = nc.dram_tensor(
        "x", [XIN_ROWS, C, W], mm_dtype, kind="ExternalInput"
    ).ap()
    wt = nc.dram_tensor("wt", [3, 128, 96], mm_dtype, kind="ExternalInput").ap()
    # round-major output: psum partition 32g+j of round r is unit u=4r+g,
    # M-row j -- a whole round's drained output [128, WOUT] is one
    # contiguous 393KB block. One clean 2D DMA per round; the host
    # unscrambles u = 3w+s -> (co, r, row).
    y = nc.dram_tensor(
        "y", [NROUNDS, 128, WOUT], F16, kind="ExternalOutput"
    ).ap()

    # input tiles batch several windows: [128, nw, W] with window slot i
    # holding rows 6w..6w+7 at p = k*16+ci. Fresh rows (k=2..7) of all
    # slots arrive as ONE DMA (~1.18MB for nw=4 -> line-rate HBM); halo
    # rows (k=0,1) are partition-shift DVE copies from the previous slot's
    # k=6,7. The first two tiles are small (1+3 windows) so the PE can
    # start after a 393KB load instead of 1.3MB.
    tile_specs = [(0, 1), (1, 3)] + [(w0, 4) for w0 in range(4, NBLK, 4)]

    with tile.TileContext(nc) as tc:
        with (
            tc.tile_pool(name="wpool", bufs=1) as wpool,
            tc.tile_pool(name="winp", bufs=wbufs) as winp,
            tc.tile_pool(name="opool", bufs=obufs) as opool,
            tc.tile_pool(name="ppool", bufs=pbufs, space="PSUM") as ppool,
        ):
            wts = []
            for dx in range(3):
                wtile = wpool.tile([128, 96], mm_dtype, tag=f"w{dx}", name=f"w{dx}")
                nc.sync.dma_start(wtile[:], wt[dx])
                wts.append(wtile)

            win_slot = {}  # window -> (tile, slot)
            tiles = {}

            def emit_fresh(ti):
                w0, nw = tile_specs[ti]
                t = winp.tile([128, nw, W], mm_dtype, tag="win", name=f"wq{w0}")
                if w0 == 0:
                    # rows 0..7 (incl. halo) in one full-height load
                    nc.sync.dma_start(
                        t[:, 0, :], x[0:WIN].rearrange("r c x -> (r c) x")
                    )
                else:
                    # fresh rows k=2..7 for every slot: src row =
                    # 6*(w0+slot)+2+kk -> 4D AP (kk, ci, slot, x)
                    nc.sync.dma_start(
                        t[32:128, :, :],
                        x[R * w0 + 2 : R * (w0 + nw) + 2]
                        .rearrange("(s r) c x -> (r c) s x", s=nw)
                    )
                for i in range(nw):
                    win_slot[w0 + i] = (t, i)
                tiles[ti] = (t, nw)

            def emit_halos(ti):
                # Halo copies go on DVE, but are emitted 1 quad before
                # first use while their fresh DMA was issued 2 quads
                # ahead: by the time a halo reaches the strict-FIFO DVE
                # queue head its dependency is met, so it never blocks
                # the PSUM drains queued behind it. (GpSimd copies cost
                # 5.3us each; ACT 1.6us; DVE 0.55us.)
                w0, nw = tile_specs[ti]
                t, _ = tiles[ti]
                if w0 != 0:
                    pt, pnw = tiles[ti - 1]
                    nc.vector.tensor_copy(
                        t[0:32, 0, :], pt[96:128, pnw - 1, :]
                    )
                for i in range(1, nw):
                    nc.vector.tensor_copy(t[0:32, i, :], t[96:128, i - 1, :])

            emit_fresh(0)
            emit_fresh(1)
            emit_halos(0)
            emit_halos(1)
            next_fresh = 2
            next_halo = 2
            for r in range(NROUNDS):
                wmax = (4 * r + 3) // 3
                while next_fresh < len(tile_specs) and (
                    tile_specs[next_fresh][0] <= wmax + 2 * prefetch
                ):
                    emit_fresh(next_fresh)
                    next_fresh += 1
                # halos only 1 window ahead of use: any earlier and they
                # sit at the DVE queue head waiting on their quad's DMA,
                # blocking every drain queued behind them
                while next_halo < next_fresh and (
                    tile_specs[next_halo][0] <= wmax + 1
                ):
                    emit_halos(next_halo)
                    next_halo += 1
                psums = [
                    ppool.tile([128, 512], F32, tag="ps", name=f"ps{r}_{c}")
                    for c in range(len(CHUNKS))
                ]
                for dx in range(3):
                    for g in range(4):
                        u = 4 * r + g
                        w_, s = u // 3, u % 3
                        t, slot = win_slot[w_]
                        for c, (x0, n) in enumerate(CHUNKS):
                            nc.tensor.matmul(
                                psums[c][32 * g : 32 * g + 32, :n],
                                wts[dx][:, 32 * s : 32 * s + 32],
                                t[:, slot, x0 + dx : x0 + dx + n],
                                start=(dx == 0),
                                stop=(dx == 2),
                                tile_position=(0, 32 * g),
                            )
                ot = opool.tile([128, WOUT], F16, tag="o", name=f"o{r}")
                for c, (x0, n) in enumerate(CHUNKS):
                    eng_add = (
                        nc.vector.tensor_scalar_add
                        if (r + c) % 2 == 0
                        else nc.scalar.add
                    )
                    eng_add(
                        ot[:, x0 : x0 + n], psums[c][:, :n], float(bias_sum)
                    )
                # scalar ring only: the input ring must never block behind
                # output WARs (a blocked DMA issue stalls its whole queue)
                nc.scalar.dma_start(y[r], ot[:])
    return nc


def kernel(X: np.ndarray, weight: np.ndarray, bias: np.ndarray) -> np.ndarray:
    nc, in_maps, starts = prepare(X, weight, bias)
    # the device occasionally faults transiently (NRT_EXEC_UNIT_UNRECOVERABLE)
    # -- retry a couple of times before giving up
    last_err = None
    for _ in range(3):
        try:
            res = run_bass_kernel_spmd(
                nc, in_maps, core_ids=list(range(NCORES))
            )
            break
        except Exception as e:  # noqa: BLE001
            last_err = e
    else:
        raise last_err

    return assemble(res.results, starts)



# revision 34
# speedup vs baseline: 1.0575x; 1.0575x over previous
"""Trainium2 Bass kernel for 16->16 channel 3x3 VALID conv on [16,1536,1536].

out[co, y, x] = sum_{ci,dy,dx} W[co,ci,dy,dx] * X[ci, y+dy, x+dx] + sum(bias)

Strategy (8-core data parallel over H, halo of 2 rows):
  Each core computes 192 output rows from a 194-row input shard, in 32 blocks
  of R=6 output rows. Per block, an SBUF "window" of 8 input rows x 16
  channels is laid out as [128, 1536] with partition p = k*16 + ci (k = row
  in window, k-major). The conv becomes 3 accumulating matmuls (one per
  kernel column dx) whose dx shift is a free-dim offset on the moving
  operand:
     psum[m=(co*6+r), x] += lhsT_dx[p, m] * window[p, x+dx]
  with block-Toeplitz weights lhsT_dx[k*16+ci, co*6+r] = W[co,ci,k-r,dx]
  (zero outside 0 <= k-r <= 2), precomputed on host from the 9KB weight.
  Contraction K=128, M=96 -> 3 column-streams per 6 output pixels (0.5
  PE-cycles/output-pixel).

  Matmuls run in float32r (tf32-like, 1 col/cycle at N>=256; fp32 would be
  4x slower). The rounding to f32r happens in a Vector-engine copy; the
  verifier requires a rounding producer for f32r matmul inputs.

  HBM traffic: the 2 halo rows shared by consecutive blocks are carried
  across in SBUF (window k=6,7 -> next window k=0,1 via a DVE copy), so each
  input row is read exactly once. Input DMAs issue on the sync-engine HWDGE
  queue, output DMAs on the scalar-engine queue to avoid queue serialization.
"""

import numpy as np

import concourse.bass as bass
import concourse.mybir as mybir
import concourse.tile as tile
from concourse.bass_utils import run_bass_kernel_spmd

C = 16
H = 1536
W = 1536
HOUT = H - 2
WOUT = W - 2
NCORES = 8
ROWS_PER_CORE = 192  # output rows computed per core
R = 6                # output rows per block
WIN = R + 2          # input rows per window
NBLK = ROWS_PER_CORE // R
XIN_ROWS = ROWS_PER_CORE + 2  # input rows per shard
CHUNKS = [(0, 512), (512, 512), (1024, WOUT - 1024)]
F32 = mybir.dt.float32
F32R = mybir.dt.float32r
F16 = mybir.dt.float16

_drain_patched = False


def _patch_tile_drain():
    """This container's walrus accepts only ONE sync-wait per lowered
    instruction (CTRL drains, S3_LW weight loads, ...). Tile freely attaches
    several. Split the extras onto single-wait nops placed just before the
    instruction on the same engine (identical blocking semantics)."""
    global _drain_patched
    if _drain_patched:
        return
    _drain_patched = True
    from concourse.tile import ScopedClock

    def _split_multi_waits(ordered):
        for bb_name, insts in ordered.items():
            out = []
            for inst in insts:
                si = getattr(inst, "sync_info", None)
                if (
                    si is not None
                    and si.on_wait is not None
                    and len(si.on_wait) > 1
                    and type(inst).__module__ == "bass_rust"
                ):
                    waits = list(si.on_wait)
                    for i, w in enumerate(waits[:-1]):
                        out.append(
                            mybir.InstNoOp(
                                name=f"{inst.name}ws{i}",
                                engine=inst.engine,
                                bass_nofuse=True,
                                sync_info=mybir.SyncInfo(
                                    on_wait=[w], on_update=[]
                                ),
                            )
                        )
                    inst.sync_info = mybir.SyncInfo(
                        on_wait=[waits[-1]],
                        on_update=list(si.on_update or []),
                    )
                out.append(inst)
            ordered[bb_name] = out
        return ordered

    orig_lower = tile.TileContext._lower_ordered_insts

    def _lower_ordered_insts(self, ordered):
        return orig_lower(self, _split_multi_waits(ordered))

    tile.TileContext._lower_ordered_insts = _lower_ordered_insts

    def _drain_and_barrier(self, tick_clock, wait_clock):
        drain_inst = self.nc.sync.drain()
        wait_clock.add_sem_waits(
            drain_inst.ins, ScopedClock({None: tick_clock.global_clock})
        )
        si = drain_inst.ins.sync_info
        if si is not None and si.on_wait is not None and len(si.on_wait) > 1:
            waits = list(si.on_wait)
            drain_inst.ins.sync_info = mybir.SyncInfo(
                on_wait=[waits[0]], on_update=list(si.on_update or [])
            )
            for w in waits[1:]:
                n = self.nc.sync.nop(nofuse=True, hint="drain_wait_split")
                n.ins.sync_info = mybir.SyncInfo(on_wait=[w], on_update=[])
        self.nc.all_engine_barrier()
        assert self.sems is not None
        popped = self.nc._tile_sem_poison_stack.pop()
        assert popped is self._sem_poison
        self.nc.clear_and_free_semaphores(list(self.sems.allocated().values()))
        self.nc.all_engine_barrier()

    tile.TileContext._drain_and_barrier = _drain_and_barrier


def build_lhsT(weight: np.ndarray) -> np.ndarray:
    """[C_out=16, C_in=16, 3, 3] -> [3, 128, 96] block-Toeplitz stationary
    operands, one per kernel column dx.
    lhsT[dx, ci*8+k, co*6+r] = weight[co, ci, k-r, dx] for 0 <= k-r <= 2."""
    lhsT = np.zeros((3, 128, 96), np.float32)
    ci = np.arange(C)
    co = np.arange(C)
    for dx in range(3):
        for dy in range(3):
            for r in range(R):
                k = r + dy
                lhsT[dx, (ci * WIN + k)[:, None], (co * R + r)[None, :]] = (
                    weight[:, :, dy, dx].T
                )
    return lhsT


def shard_windows(
    Xs: np.ndarray, group: int, dtype=np.float32
) -> np.ndarray:
    """Host-side window predup for one core's shard [C, XIN_ROWS, W] ->
    [ngroups, 128, group*W] where out[g, ci*8+k, w*W+x] =
    Xs[ci, 6*(group*g+w)+k, x]. dma_start carries a large fixed overhead
    here, so shipping the 33% halo duplication in exchange for one
    contiguous multi-MB DMA per group is a clear win. For 16-bit matmul
    dtypes the cast happens here too, halving the DMA bytes."""
    ngroups = NBLK // group
    rows = (
        R * group * np.arange(ngroups)[:, None, None]
        + R * np.arange(group)[None, :, None]
        + np.arange(WIN)[None, None, :]
    )  # [g, w, k]
    arr = Xs[:, rows, :].astype(dtype)  # [C, g, w, k, W]
    arr = arr.transpose(1, 0, 3, 2, 4)  # [g, C, k, w, W]
    return np.ascontiguousarray(
        arr.reshape(ngroups, 128, group * W)
    )


def build_program(
    bias_sum: float,
    mm_dtype=F32R,
    nblk=NBLK,
    group=4,
    xbufs=3,
    wbufs=2,
    obufs=4,
    pbufs=6,
    repeat=1,
    hw_loop=0,
):
    """One core's program: see module docstring. dma_start carries a large
    fixed overhead in this environment, so inputs arrive as host-preduped
    window groups (one contiguous DMA per `group` blocks) and outputs leave
    as one grouped DMA into a [C, R, nblk, WOUT] device layout the host
    re-transposes. `repeat` (python-unrolled) and `hw_loop` (tc.For_i)
    re-run the whole block sweep, for timing amplification only."""
    nc = bass.Bass("TRN2", target_bir_lowering=False, debug=False)
    assert nblk % group == 0
    ngroups = nblk // group
    # 16-bit matmul dtypes are cast host-side: x/wt ship pre-cast, halving
    # input DMA bytes and skipping the on-chip rounding pass. f32r still
    # needs an on-chip DVE rounding producer.
    host_cast = mybir.dt.size(mm_dtype) == 2
    ship_dtype = mm_dtype if host_cast else F32
    x = nc.dram_tensor(
        "x", [ngroups, 128, group * W], ship_dtype, kind="ExternalInput"
    ).ap()
    wt = nc.dram_tensor(
        "wt", [3, 128, 96], ship_dtype, kind="ExternalInput"
    ).ap()
    # f16 output halves the store-side HBM traffic; the host re-expands to
    # f32. Quantization error (~2^-11 of max|out|) is far inside the 2e-2
    # rel-err budget.
    y = nc.dram_tensor(
        "y", [C, R, nblk, WOUT], F16, kind="ExternalOutput"
    ).ap()
    round_on_chip = (not host_cast) and mm_dtype != F32

    with tile.TileContext(nc) as tc:
        with (
            tc.tile_pool(name="wpool", bufs=1) as wpool,
            tc.tile_pool(name="xpool", bufs=xbufs) as xpool,
            tc.tile_pool(name="winp", bufs=wbufs) as winp,
            tc.tile_pool(name="opool", bufs=obufs) as opool,
            tc.tile_pool(name="ppool", bufs=pbufs, space="PSUM") as ppool,
        ):
            wts = []
            for dx in range(3):
                if round_on_chip:
                    ws = wpool.tile([128, 96], F32, tag=f"ws{dx}", name=f"ws{dx}")
                    nc.sync.dma_start(ws[:], wt[dx])
                    wtile = wpool.tile(
                        [128, 96], mm_dtype, tag=f"w{dx}", name=f"w{dx}"
                    )
                    nc.vector.tensor_copy(wtile[:], ws[:])
                else:
                    wtile = wpool.tile(
                        [128, 96], mm_dtype, tag=f"w{dx}", name=f"w{dx}"
                    )
                    nc.sync.dma_start(wtile[:], wt[dx])
                wts.append(wtile)

            def sweep(prefix):
                for g in range(ngroups):
                    uid = f"{prefix}_{g}"
                    # one contiguous DMA loads `group` preduped 8-row
                    # windows; partition p = ci*8+k, window w at free cols
                    # [w*W, (w+1)*W)
                    if round_on_chip:
                        stage = xpool.tile(
                            [128, group, W], F32, tag="stage", name=f"st{uid}"
                        )
                        nc.sync.dma_start(
                            stage[:],
                            x[g].rearrange("p (w c) -> p w c", w=group),
                        )
                        win = winp.tile(
                            [128, group, W], mm_dtype, tag="win",
                            name=f"win{uid}",
                        )
                        nc.vector.tensor_copy(win[:], stage[:])
                    else:
                        win = winp.tile(
                            [128, group, W], mm_dtype, tag="win",
                            name=f"win{uid}",
                        )
                        nc.sync.dma_start(
                            win[:],
                            x[g].rearrange("p (w c) -> p w c", w=group),
                        )

                    ot = opool.tile(
                        [96, group, W], F16, tag="o", name=f"o_{uid}"
                    )
                    for w in range(group):
                        psums = [
                            ppool.tile(
                                [96, 512], F32, tag="ps", name=f"ps_{uid}_{w}_{i}"
                            )
                            for i in range(len(CHUNKS))
                        ]
                        for dx in range(3):
                            for ic, (x0, n) in enumerate(CHUNKS):
                                nc.tensor.matmul(
                                    psums[ic][:, :n],
                                    wts[dx][:],
                                    win[:, w, x0 + dx : x0 + dx + n],
                                    start=(dx == 0),
                                    stop=(dx == 2),
                                )
                        # PSUM drain + bias + f16 cast on DVE (~680ns per
                        # [96,512] chunk).
                        for ic, (x0, n) in enumerate(CHUNKS):
                            nc.vector.tensor_scalar_add(
                                ot[:, w, x0 : x0 + n],
                                psums[ic][:, :n],
                                float(bias_sum),
                            )
                    # one DMA stores `group` blocks of 6 output rows into the
                    # [C, R, nblk, WOUT] device layout; (blk, x) merge keeps
                    # the dest AP 3-dim. scalar-engine HWDGE queue keeps
                    # outputs off the input queue.
                    nc.scalar.dma_start(
                        y[:, :, g * group : (g + 1) * group, :].rearrange(
                            "c r b x -> c r (b x)"
                        ),
                        ot[:96, :, 0:WOUT],
                    )

            if hw_loop:
                with tc.For_i(
                    0, hw_loop, 1, hint_engines=(mybir.EngineType.PE,)
                ):
                    sweep("L")
            else:
                for rep in range(repeat):
                    sweep(str(rep))
    return nc


def prepare(X: np.ndarray, weight: np.ndarray, bias: np.ndarray):
    """Build (nc, in_maps, starts) for the exact program kernel() runs."""
    X = np.ascontiguousarray(np.asarray(X, dtype=np.float32))
    weight = np.asarray(weight, dtype=np.float32)
    bias = np.asarray(bias, dtype=np.float32)

    _patch_tile_drain()
    import os

    starts = [min(c * ROWS_PER_CORE, H - XIN_ROWS) for c in range(NCORES)]
    if os.environ.get("CONV_V", "2") == "2":
        lhsT = build_lhsT2(weight).astype(np.float16)
        nc = build_program2(float(bias.sum()))
        in_maps = [
            {
                # [194, 16, W] row-major f16 raw shard
                "x": np.ascontiguousarray(
                    X[:, s : s + XIN_ROWS, :].transpose(1, 0, 2)
                ).astype(np.float16),
                "wt": lhsT,
            }
            for s in starts
        ]
        return nc, in_maps, starts

    lhsT = build_lhsT(weight)
    mm_dtype = {
        "f32": F32,
        "f32r": F32R,
        "f16": mybir.dt.float16,
        "bf16": mybir.dt.bfloat16,
    }[os.environ.get("CONV_MM_DTYPE", "f16")]
    group = 4
    host_cast = mybir.dt.size(mm_dtype) == 2
    ship = mybir.dt.np(mm_dtype) if host_cast else np.float32
    wbufs = 6 if host_cast else 2
    nc = build_program(
        float(bias.sum()), mm_dtype, group=group, wbufs=wbufs
    )
    in_maps = [
        {
            "x": shard_windows(X[:, s : s + XIN_ROWS, :], group, ship),
            "wt": lhsT.astype(ship),
        }
        for s in starts
    ]
    return nc, in_maps, starts


def assemble(res_results, starts) -> np.ndarray:
    out = np.empty((C, HOUT, WOUT), np.float32)
    for c in range(NCORES):
        yc = np.asarray(res_results[c]["y"], np.float32)
        if yc.shape[0] == NROUNDS:
            # v2 round-major [NROUNDS, 128, WOUT]: flat row u*32+j with
            # u = 3w+s holding M-rows m = 32s+j of window w (m = co*6+r)
            yc = (
                yc.reshape(NBLK, 3, 32, WOUT)
                .transpose(1, 2, 0, 3)
                .reshape(C, R, NBLK, WOUT)
            )
        out[:, starts[c] : starts[c] + ROWS_PER_CORE, :] = (
            yc.transpose(0, 2, 1, 3).reshape(C, ROWS_PER_CORE, WOUT)
        )
    return out


def build_lhsT2(weight: np.ndarray) -> np.ndarray:
    """k-major block-Toeplitz: lhsT[dx, k*16+ci, co*6+r] = W[co,ci,k-r,dx]
    for 0 <= k-r <= 2. k-major partitions put the window's two halo rows
    (k=6,7) at partitions 96:128 so the next window's halo lands at 0:32
    with a single contiguous partition-shift copy."""
    lhsT = np.zeros((3, 128, 96), np.float32)
    ci = np.arange(C)
    co = np.arange(C)
    for dx in range(3):
        for dy in range(3):
            for r in range(R):
                k = r + dy
                lhsT[dx, (k * 16 + ci)[:, None], (co * R + r)[None, :]] = (
                    weight[:, :, dy, dx].T
                )
    return lhsT


NUNITS = 3 * NBLK  # 96 M=32 col-tile units per core
NROUNDS = NUNITS // 4  # 24 rounds of 4 concurrent units


def build_program2(
    bias_sum: float,
    mm_dtype=F16,
    wbufs=6,
    obufs=6,
    pbufs=8,
    prefetch=4,
):
    """Raw-input, col-tiled program (one core).

    Input x: [194, 16, W] f16 row-major. Window w lives in an SBUF tile
    [128, W], partition p = k*16+ci with k = row-6w. Fresh rows 6w+2..6w+7
    DMA into partitions 32:128 (one contiguous 294KB HBM read); rows
    6w..6w+1 equal window w-1's FRESH rows k=6,7 (partitions 96:128), so a
    single partition-shift DVE copy [32, W] fills 0:32 — halo copies do
    not chain. Input HBM traffic is the raw 9.53MB/core (no 33% predup).

    Matmuls are 4x column-tiled: unit u = 3w+s (s = 32-wide M-slice of
    window w) runs on PE column group g = u%4 via tile_position=(0,32g),
    writing psum quarter [32g:32g+32]. Four units stream concurrently
    through separate XBUSes (HW-probed ~4x), cutting PE time from ~65us
    (M=96, 3/4 of the array) to ~48us. Per round (4 units) there are 3
    chunk psums [128,512] accumulated over dx; drains (+bias, ->f16)
    alternate DVE/ACT per chunk; 2 output DMAs per round go to
    y[96, nblk, WOUT] (m = co*6+r), queues alternating scalar/sync.
    """
    nc = bass.Bass("TRN2", target_bir_lowering=False, debug=False)
    x = nc.dram_tensor(
        "x", [XIN_ROWS, C, W], mm_dtype, kind="ExternalInput"
    ).ap()
    wt = nc.dram_tensor("wt", [3, 128, 96], mm_dtype, kind="ExternalInput").ap()
    # round-major output: psum partition 32g+j of round r is unit u=4r+g,
    # M-row j -- a whole round's drained output [128, WOUT] is one
    # contiguous 393KB block. One clean 2D DMA per round; the host
    # unscrambles u = 3w+s -> (co, r, row).
    y = nc.dram_tensor(
        "y", [NROUNDS, 128, WOUT], F16, kind="ExternalOutput"
    ).ap()

    # input tiles batch several windows: [128, nw, W] with window slot i
    # holding rows 6w..6w+7 at p = k*16+ci. Fresh rows (k=2..7) of all
    # slots arrive as ONE DMA (~1.18MB for nw=4 -> line-rate HBM); halo
    # rows (k=0,1) are partition-shift DVE copies from the previous slot's
    # k=6,7. The first two tiles are small (1+3 windows) so the PE can
    # start after a 393KB load instead of 1.3MB.
    tile_specs = [(0, 1), (1, 3)] + [(w0, 4) for w0 in range(4, NBLK, 4)]

    with tile.TileContext(nc) as tc:
        with (
            tc.tile_pool(name="wpool", bufs=1) as wpool,
            tc.tile_pool(name="winp", bufs=wbufs) as winp,
            tc.tile_pool(name="opool", bufs=obufs) as opool,
            tc.tile_pool(name="ppool", bufs=pbufs, space="PSUM") as ppool,
        ):
            wts = []
            for dx in range(3):
                wtile = wpool.tile([128, 96], mm_dtype, tag=f"w{dx}", name=f"w{dx}")
                nc.sync.dma_start(wtile[:], wt[dx])
                wts.append(wtile)

            win_slot = {}  # window -> (tile, slot)
            tiles = {}

            def emit_fresh(ti):
                w0, nw = tile_specs[ti]
                t = winp.tile([128, nw, W], mm_dtype, tag="win", name=f"wq{w0}")
                if w0 == 0:
                    # rows 0..7 (incl. halo) in one full-height load
                    nc.sync.dma_start(
                        t[:, 0, :], x[0:WIN].rearrange("r c x -> (r c) x")
                    )
                else:
                    # fresh rows k=2..7 for every slot: src row =
                    # 6*(w0+slot)+2+kk -> 4D AP (kk, ci, slot, x)
                    nc.sync.dma_start(
                        t[32:128, :, :],
                        x[R * w0 + 2 : R * (w0 + nw) + 2]
                        .rearrange("(s r) c x -> (r c) s x", s=nw)
                    )
                for i in range(nw):
                    win_slot[w0 + i] = (t, i)
                tiles[ti] = (t, nw)

            def emit_halos(ti):
                # Halo copies go on DVE, but are emitted 1 quad before
                # first use while their fresh DMA was issued 2 quads
                # ahead: by the time a halo reaches the strict-FIFO DVE
                # queue head its dependency is met, so it never blocks
                # the PSUM drains queued behind it. (GpSimd copies cost
                # 5.3us each; ACT 1.6us; DVE 0.55us.)
                w0, nw = tile_specs[ti]
                t, _ = tiles[ti]
                if w0 != 0:
                    pt, pnw = tiles[ti - 1]
                    nc.vector.tensor_copy(
                        t[0:32, 0, :], pt[96:128, pnw - 1, :]
                    )
                for i in range(1, nw):
                    nc.vector.tensor_copy(t[0:32, i, :], t[96:128, i - 1, :])

            emit_fresh(0)
            emit_fresh(1)
            emit_halos(0)
            emit_halos(1)
            # ~3.5us of dummy matmuls on the (tiny, early-landing) weight
            # tiles while the first input quads stream in: flips the PE's
            # HAM clock-gate to 8/8 before round 0, which otherwise runs
            # its first ~3.4us at 1.2GHz and starts the drain pipeline
            # ~4us late (a lag the round->drain->round coupling never
            # recovers).
            wu = ppool.tile([128, 512], F32, tag="ps", name="warm")
            for i in range(36):
                nc.tensor.matmul(
                    wu[0:96, 0:96], wts[0][:], wts[1][:], start=True, stop=True
                )
            next_fresh = 2
            next_halo = 2
            for r in range(NROUNDS):
                wmax = (4 * r + 3) // 3
                while next_fresh < len(tile_specs) and (
                    tile_specs[next_fresh][0] <= wmax + 2 * prefetch
                ):
                    emit_fresh(next_fresh)
                    next_fresh += 1
                # halos only 1 window ahead of use: any earlier and they
                # sit at the DVE queue head waiting on their quad's DMA,
                # blocking every drain queued behind them
                while next_halo < next_fresh and (
                    tile_specs[next_halo][0] <= wmax + 1
                ):
                    emit_halos(next_halo)
                    next_halo += 1
                psums = [
                    ppool.tile([128, 512], F32, tag="ps", name=f"ps{r}_{c}")
                    for c in range(len(CHUNKS))
                ]
                # chunk-outermost: chunk c's 12 MMs finish first, so its
                # drain overlaps the round's remaining chunks instead of
                # only starting after the whole round
                for c, (x0, n) in enumerate(CHUNKS):
                    for dx in range(3):
                        for g in range(4):
                            u = 4 * r + g
                            w_, s = u // 3, u % 3
                            t, slot = win_slot[w_]
                            nc.tensor.matmul(
                                psums[c][32 * g : 32 * g + 32, :n],
                                wts[dx][:, 32 * s : 32 * s + 32],
                                t[:, slot, x0 + dx : x0 + dx + n],
                                start=(dx == 0),
                                stop=(dx == 2),
                                tile_position=(0, 32 * g),
                            )
                ot = opool.tile([128, WOUT], F16, tag="o", name=f"o{r}")
                for c, (x0, n) in enumerate(CHUNKS):
                    eng_add = (
                        nc.vector.tensor_scalar_add
                        if (r + c) % 2 == 0
                        else nc.scalar.add
                    )
                    eng_add(
                        ot[:, x0 : x0 + n], psums[c][:, :n], float(bias_sum)
                    )
                # scalar ring only: the input ring must never block behind
                # output WARs (a blocked DMA issue stalls its whole queue)
                nc.scalar.dma_start(y[r], ot[:])
    return nc


def kernel(X: np.ndarray, weight: np.ndarray, bias: np.ndarray) -> np.ndarray:
    nc, in_maps, starts = prepare(X, weight, bias)
    # the device occasionally faults transiently (NRT_EXEC_UNIT_UNRECOVERABLE)
    # -- retry a couple of times before giving up
    last_err = None
    for _ in range(3):
        try:
            res = run_bass_kernel_spmd(
                nc, in_maps, core_ids=list(range(NCORES))
            )
            break
        except Exception as e:  # noqa: BLE001
            last_err = e
    else:
        raise last_err

    return assemble(res.results, starts)

